# revision 1
# baseline (speedup 1.0000x reference)
"""Trainium2 Bass kernel for nn_CausalAttn_24618752541290.

Causal attention: B=2, L=2048, D=2048, H=16 heads, Dh=128, with RoPE
(theta=5e5, interleaved pairs), QK L2-normalization, causal softmax with a
runtime scale, and an output projection.

Sharding (8 NeuronCores): data-parallel over batch (2) x tensor-parallel over
head groups (4 heads/core).  Core i handles batch i//4 and heads
[4*(i%4), 4*(i%4)+4).  Each core computes a partial [L, D] output
(row-parallel Wout over its head slice); the host sums the 4 partials per
batch.

Device math (per core), validated against the reference in fp64:
  - host permutes Wq/Wk columns within each head (pairs->halves) so RoPE
    becomes the rotate-half form; scores are invariant to the common
    permutation of q and k.
  - rope(t) = t*C + swap64(t)*S with host-precomputed C/S tables.
  - L2 norm is rotation-invariant -> computed pre-rope.  q is normalized
    explicitly; the k-side 1/(||k||+eps) factor (times attn_scale) is folded
    into the per-partition `scale` argument of the exp activation.
  - no-max softmax: q,k unit vectors => |scores| <= 1, so exp(scale*s) with
    |scale| <= ~80 cannot overflow fp32; denominators accumulate in PSUM.
  - scores are built transposed (S^T[lk, lq]) so the P^T blocks feed the
    attn@v matmul directly; row sums (denominators) come from a ones-vector
    matmul; causal masking is multiplicative on the 4 diagonal blocks.
"""

import os
import sys

for p in ("/opt/trn_rl_repo",):
    if p not in sys.path:
        sys.path.insert(0, p)

import numpy as np
import ml_dtypes

import concourse.bass as bass
import concourse.mybir as mybir
from concourse.tile import TileContext
from contextlib import ExitStack

B, L, D, H = 2, 2048, 2048, 16
Dh = 128
NH = 4              # heads per core
N_CORES = 8
EPS = 1e-6
THETA = 500000.0

F32 = mybir.dt.float32
F16 = mybir.dt.float16
BF16 = mybir.dt.bfloat16


# ---------------------------------------------------------------------------
# workaround: this container's walrus build rejects CTRL instructions (Drain)
# carrying more than one semaphore wait ("Too many sync wait commands").  The
# TileContext exit drain waits on every DMA-HW queue used by the kernel, so
# split the waits across a chain of drains, one wait each.
# ---------------------------------------------------------------------------

def _split_drain_and_barrier(self, tick_clock, wait_clock):
    from concourse.vector_clock import ScopedClock
    import bass_rust

    drain_inst = self.nc.sync.drain()
    wait_clock.add_sem_waits(
        drain_inst.ins, ScopedClock({None: tick_clock.global_clock}))
    si = drain_inst.ins.sync_info
    if si is not None and si.on_wait is not None and len(si.on_wait) > 1:
        waits = list(si.on_wait)
        si.on_wait = waits[:1]
        for w in waits[1:]:
            extra = self.nc.sync.drain()
            esi = extra.ins.sync_info
            if esi is None:
                extra.ins.sync_info = bass_rust.SyncInfo(
                    on_wait=[w], on_update=[])
            else:
                esi.on_wait = [w]

    self.nc.all_engine_barrier()
    assert self.sems is not None
    popped = self.nc._tile_sem_poison_stack.pop()
    assert popped is self._sem_poison
    self.nc.clear_and_free_semaphores(list(self.sems.allocated().values()))
    self.nc.all_engine_barrier()


TileContext._drain_and_barrier = _split_drain_and_barrier


# ---------------------------------------------------------------------------
# workaround #2: the same walrus build rejects ANY instruction carrying more
# than one semaphore wait.  Tile's add_semaphores pass freely attaches 2-4.
# Rewrite the serialized BIR: for each instruction with k>1 waits, insert k-1
# pure-wait EventSemaphore instructions (same engine) immediately before it.
# ---------------------------------------------------------------------------

def _split_waits_json(mod: dict, max_waits: int = 1) -> dict:
    for fn in mod.get("functions", []):
        for bb in fn.get("blocks", []):
            out = []
            for inst in bb.get("instructions", []):
                si = inst.get("sync_info")
                waits = (si or {}).get("on_wait") or []
                if len(waits) > max_waits:
                    extra, keep = waits[:-max_waits], waits[-max_waits:]
                    for k_, w in enumerate(extra):
                        out.append({
                            "debug": inst.get("debug", 0),
                            "engine": inst["engine"],
                            "ins": [],
                            "name": f"{inst['name']}.wsplit{k_}",
                            "opcode": "EventSemaphore",
                            "outs": [],
                            "sync_info": {"on_update": [], "on_wait": [w]},
                        })
                    si["on_wait"] = keep
                out.append(inst)
            bb["instructions"] = out
    return mod


_orig_to_json_bytes = bass.Bass.to_json_bytes


def _to_json_bytes_split(self):
    import orjson
    mod = orjson.loads(_orig_to_json_bytes(self))
    _split_waits_json(mod)
    return orjson.dumps(mod)


bass.Bass.to_json_bytes = _to_json_bytes_split


# ---------------------------------------------------------------------------
# device program
# ---------------------------------------------------------------------------

def build_nc(l=L, d=D, nh=NH, dt16=F16, reps=1):
    """Build the per-core Bass program (identical on all cores; SPMD).

    reps > 1 wraps the whole body in a hardware For_i loop — used only for
    wall-clock timing (the axon dispatch floor is ~100 ms, so a single
    ~0.4 ms kernel execution is unmeasurable without on-device repetition).
    """
    KT = d // 128          # contraction tiles over D
    CH = l // 512          # L chunks of 512
    NB = l // 128          # L blocks of 128
    nc = bass.Bass()

    xT_d = nc.dram_tensor("xT", [d, l], dt16, kind="ExternalInput")
    wq_d = nc.dram_tensor("wq", [d, nh * 128], dt16, kind="ExternalInput")
    wk_d = nc.dram_tensor("wk", [d, nh * 128], dt16, kind="ExternalInput")
    wv_d = nc.dram_tensor("wv", [d, nh * 128], dt16, kind="ExternalInput")
    wo_d = nc.dram_tensor("wo", [nh * 128, d], dt16, kind="ExternalInput")
    cs_d = nc.dram_tensor("cs", [128, l], dt16, kind="ExternalInput")
    sn_d = nc.dram_tensor("sn", [128, l], dt16, kind="ExternalInput")
    mk_d = nc.dram_tensor("mk", [4, 128, 512], BF16, kind="ExternalInput")
    sc_d = nc.dram_tensor("sc", [128, 1], F32, kind="ExternalInput")
    sw_d = nc.dram_tensor("sw", [128, 128], dt16, kind="ExternalInput")
    y_d = nc.dram_tensor("y", [l, d], dt16, kind="ExternalOutput")

    xT_r = xT_d.rearrange("(kt p) l -> p kt l", p=128)
    wq_r = wq_d.rearrange("(kt p) m -> p kt m", p=128)
    wk_r = wk_d.rearrange("(kt p) m -> p kt m", p=128)
    wv_r = wv_d.rearrange("(kt p) m -> p kt m", p=128)
    wo_r = wo_d.rearrange("(h p) d -> p h d", p=128)
    mk_r = mk_d.rearrange("j p f -> p j f")
    y_r = y_d.rearrange("(m p) d -> p m d", p=128)

    Exp = mybir.ActivationFunctionType.Exp

    def bcast_ap(ap, n=128):
        # view a [1, F] SBUF tile as [n, F] with partition step 0 (DMA source)
        return bass.AP(ap.tensor, ap.offset, [[0, n]] + list(ap.ap[1:]))

    with TileContext(nc) as tc, ExitStack() as top:
        persist = top.enter_context(tc.tile_pool(name="persist", bufs=1))
        scratch = top.enter_context(tc.tile_pool(name="scratch", bufs=3))
        scr32 = top.enter_context(tc.tile_pool(name="scr32", bufs=3))
        tiny = top.enter_context(tc.tile_pool(name="tiny", bufs=2))
        drp = top.enter_context(tc.tile_pool(name="drp", bufs=3, space="DRAM"))

        def bcast128(dst, src):
            """dst[128, F] (SBUF) <- src[1, F] (SBUF) replicated, via DRAM."""
            dr = drp.tile([1, dst.shape[-1]], src.dtype, tag="dr",
                          name="dr")
            nc.sync.dma_start(dr[:], src[:])
            nc.sync.dma_start(dst[:], bcast_ap(dr))

        # persistent SBUF
        qT_sb = persist.tile([128, nh, l], dt16)
        kT_sb = persist.tile([128, nh, l], dt16)
        outT_sb = persist.tile([128, nh, l], dt16)
        v_sb = persist.tile([128, NB, nh * 128], BF16)
        crk_sb = persist.tile([128, nh, NB], F32)
        rk_sb = persist.tile([128, nh, NB], F32)
        mk_sb = persist.tile([128, 4, 512], BF16)
        sc_sb = persist.tile([128, 1], F32)
        sw_sb = persist.tile([128, 128], dt16)
        ones16 = persist.tile([128, 1], dt16)
        onesbf = persist.tile([128, 1], BF16)

        nc.sync.dma_start(mk_sb[:], mk_r[:])
        nc.sync.dma_start(sc_sb[:], sc_d[:])
        nc.sync.dma_start(sw_sb[:], sw_d[:])
        nc.vector.memset(ones16[:], 1.0)
        nc.vector.memset(onesbf[:], 1.0)

        import contextlib
        rep_cm = tc.For_i(0, reps, 1) if reps > 1 else contextlib.nullcontext()
        top.enter_context(rep_cm)

        # ---------------- q/k projections + rope + norms ----------------
        with ExitStack() as qk_phase:
            pool_w = qk_phase.enter_context(tc.tile_pool(name="pool_w", bufs=1))
            pool_cs = qk_phase.enter_context(tc.tile_pool(name="pool_cs", bufs=1))
            xcpool = qk_phase.enter_context(tc.tile_pool(name="xcpool", bufs=2))
            psq = qk_phase.enter_context(
                tc.tile_pool(name="psq", bufs=1, space="PSUM"))
            psn = qk_phase.enter_context(
                tc.tile_pool(name="psn", bufs=2, space="PSUM"))
            wq_sb = pool_w.tile([128, KT, nh * 128], dt16)
            wk_sb = pool_w.tile([128, KT, nh * 128], dt16)
            wv_sb = pool_w.tile([128, KT, nh * 128], dt16)
            cs_sb = pool_cs.tile([128, l], dt16)
            sn_sb = pool_cs.tile([128, l], dt16)
            nc.sync.dma_start(cs_sb[:], cs_d[:])
            nc.sync.dma_start(sn_sb[:], sn_d[:])
            nc.sync.dma_start(wq_sb[:], wq_r[:])
            nc.sync.dma_start(wk_sb[:], wk_r[:])
            nc.sync.dma_start(wv_sb[:], wv_r[:])

            for c in range(CH):
                cs_ = slice(c * 512, (c + 1) * 512)
                xc = xcpool.tile([128, KT, 512], dt16, tag="xc")
                for kt in range(KT):
                    nc.sync.dma_start(xc[:, kt, :], xT_r[:, kt, cs_])
                for is_k in (False, True):
                    w_sb = wk_sb if is_k else wq_sb
                    pP_l = [psq.tile([128, 512], F32,
                                     name=f"pP{h}_{int(is_k)}", tag=f"pP{h}")
                            for h in range(nh)]
                    for kt in range(KT):
                        for h in range(nh):
                            nc.tensor.matmul(
                                pP_l[h][:],
                                lhsT=w_sb[:, kt, h * 128:(h + 1) * 128],
                                rhs=xc[:, kt, :],
                                start=(kt == 0), stop=(kt == KT - 1))
                    for h in range(nh):
                        pP = pP_l[h]
                        dst = kT_sb if is_k else qT_sb
                        qf = scratch.tile([128, 512], dt16, tag="qf")
                        nc.scalar.copy(qf[:], pP[:])
                        qsq = scratch.tile([128, 512], dt16, tag="qsq")
                        nc.vector.tensor_mul(out=qsq[:], in0=qf[:], in1=qf[:])
                        # rope: qf*C + swap64(qf)*S; swap via PE permutation
                        qwP = psn.tile([128, 512], F32, tag="qwP", bufs=2)
                        nc.tensor.matmul(qwP[:], lhsT=sw_sb[:], rhs=qf[:],
                                         start=True, stop=True)
                        qc = scratch.tile([128, 512], dt16, tag="qc")
                        qs = scratch.tile([128, 512], dt16, tag="qs")
                        nc.vector.tensor_mul(out=qc[:], in0=qf[:], in1=cs_sb[:, cs_])
                        nc.vector.tensor_mul(out=qs[:], in0=qwP[:], in1=sn_sb[:, cs_])
                        if is_k:
                            # roped, unnormalized k
                            nc.vector.tensor_add(
                                out=dst[:, h, cs_], in0=qc[:], in1=qs[:])
                            # per-128-block 1/(||k||+eps) -> rk_sb
                            nk_ps = psn.tile([128, 4], F32, tag="nk_ps", bufs=1)
                            for j in range(4):
                                nc.tensor.matmul(
                                    nk_ps[:, j:j + 1],
                                    lhsT=qsq[:, j * 128:(j + 1) * 128],
                                    rhs=ones16[:],
                                    start=True, stop=True)
                            nk = tiny.tile([128, 4], F32, tag="nkr")
                            nc.scalar.sqrt(nk[:], nk_ps[:])
                            nke = tiny.tile([128, 4], F32, tag="nkr")
                            nc.vector.tensor_scalar_add(nke[:], nk[:], EPS)
                            nc.vector.reciprocal(
                                rk_sb[:, h, c * 4:(c + 1) * 4], nke[:])
                        else:
                            nc.vector.tensor_add(out=qc[:], in0=qc[:], in1=qs[:])
                            # 1/(||q||+eps), broadcast over partitions, apply
                            nq_ps = psn.tile([1, 512], F32, tag="nq_ps", bufs=1)
                            nc.tensor.matmul(nq_ps[:], lhsT=ones16[:], rhs=qsq[:],
                                             start=True, stop=True)
                            nq = tiny.tile([1, 512], F32, tag="nqr")
                            nc.scalar.sqrt(nq[:], nq_ps[:])
                            nqe = tiny.tile([1, 512], F32, tag="nqr")
                            nc.vector.tensor_scalar_add(nqe[:], nq[:], EPS)
                            rq = tiny.tile([1, 512], F32, tag="nqr")
                            nc.vector.reciprocal(rq[:], nqe[:])
                            rqb = scr32.tile([128, 512], F32, tag="bc32")
                            bcast128(rqb, rq)
                            nc.vector.tensor_mul(
                                out=dst[:, h, cs_], in0=qc[:], in1=rqb[:])
                # v projection for this chunk's 4 L-blocks, reusing xc
                for j in range(4):
                    blk = 4 * c + j
                    vP = psq.tile([128, nh * 128], F32, tag=f"pP{j}",
                                  name=f"vP{blk}")
                    for kt in range(KT):
                        nc.tensor.matmul(
                            vP[:],
                            lhsT=xc[:, kt, j * 128:(j + 1) * 128],
                            rhs=wv_sb[:, kt, :],
                            start=(kt == 0), stop=(kt == KT - 1))
                    nc.scalar.copy(v_sb[:, blk, :], vP[:])
            # crk = attn_scale * rk  (broadcast scale along free dim)
            nc.vector.tensor_tensor(
                out=crk_sb.rearrange("p h b -> p (h b)"),
                in0=rk_sb.rearrange("p h b -> p (h b)"),
                in1=sc_sb[:].to_broadcast((128, nh * NB)),
                op=mybir.AluOpType.mult)

        # ---------------- attention ----------------
        with ExitStack() as va_phase:
            # load wo during attention
            pool_wo = va_phase.enter_context(tc.tile_pool(name="pool_wo", bufs=1))
            wo_sb = pool_wo.tile([128, nh, d], dt16)
            nc.sync.dma_start(wo_sb[:], wo_r[:])
            with ExitStack() as at_phase:
                pss = at_phase.enter_context(
                    tc.tile_pool(name="pss", bufs=4, space="PSUM"))
                psav = at_phase.enter_context(
                    tc.tile_pool(name="psav", bufs=2, space="PSUM"))
                psdn = at_phase.enter_context(
                    tc.tile_pool(name="psdn", bufs=2, space="PSUM"))
                ppool = at_phase.enter_context(tc.tile_pool(name="ppool", bufs=6))

                for h in range(nh):
                    for c in range(CH):
                        cs_ = slice(c * 512, (c + 1) * 512)
                        nbk = 4 * c + 4
                        avP = psav.tile([128, 512], F32, tag="avP")
                        dnP = psdn.tile([1, 512], F32, tag="dnP")
                        for bk in range(nbk):
                            sP = pss.tile([128, 512], F32, tag="sP")
                            nc.tensor.matmul(
                                sP[:],
                                lhsT=kT_sb[:, h, bk * 128:(bk + 1) * 128],
                                rhs=qT_sb[:, h, cs_],
                                start=True, stop=True)
                            Pb = ppool.tile([128, 512], BF16, tag="Pb")
                            nc.scalar.activation(
                                Pb[:], sP[:], Exp, scale=crk_sb[:, h, bk:bk + 1])
                            j = bk - 4 * c
                            if j >= 0:
                                nc.vector.tensor_mul(
                                    out=Pb[:], in0=Pb[:], in1=mk_sb[:, j, :])
                            nc.tensor.matmul(dnP[:], lhsT=onesbf[:], rhs=Pb[:],
                                             start=(bk == 0), stop=(bk == nbk - 1))
                            nc.tensor.matmul(
                                avP[:],
                                lhsT=v_sb[:, bk, h * 128:(h + 1) * 128],
                                rhs=Pb[:],
                                start=(bk == 0), stop=(bk == nbk - 1))
                        rdn = tiny.tile([1, 512], F32, tag="rdn")
                        nc.vector.reciprocal(rdn[:], dnP[:])
                        rdb = scr32.tile([128, 512], F32, tag="bc32")
                        bcast128(rdb, rdn)
                        nc.vector.tensor_mul(
                            out=outT_sb[:, h, cs_], in0=avP[:], in1=rdb[:])

            # ---------------- output projection ----------------
            with ExitStack() as wo_phase:
                psy = wo_phase.enter_context(
                    tc.tile_pool(name="psy", bufs=2, space="PSUM"))
                ypool = wo_phase.enter_context(tc.tile_pool(name="ypool", bufs=4))
                ND = d // 512
                for m in range(NB):
                    yP = [psy.tile([128, 512], F32, name=f"yP{n}", tag=f"yP{n % 4}")
                          for n in range(ND)]
                    for h in range(nh):
                        for n in range(ND):
                            nc.tensor.matmul(
                                yP[n][:],
                                lhsT=outT_sb[:, h, m * 128:(m + 1) * 128],
                                rhs=wo_sb[:, h, n * 512:(n + 1) * 512],
                                start=(h == 0), stop=(h == nh - 1))
                    for n in range(ND):
                        ysb = ypool.tile([128, 512], dt16, tag="ysb")
                        nc.vector.tensor_copy(out=ysb[:], in_=yP[n][:])
                        nc.sync.dma_start(y_r[:, m, n * 512:(n + 1) * 512], ysb[:])

    return nc


# ---------------------------------------------------------------------------
# host-side input marshalling
# ---------------------------------------------------------------------------

def _rope_tables(l, np16):
    inv = 1.0 / (THETA ** (np.arange(0, Dh, 2, dtype=np.float64) / Dh))
    t = np.arange(l, dtype=np.float64)
    fr = np.outer(t, inv)                       # [l, 64]
    cos, sin = np.cos(fr).T, np.sin(fr).T       # [64, l]
    C = np.concatenate([cos, cos], axis=0).astype(np16)
    S = np.concatenate([-sin, sin], axis=0).astype(np16)
    return np.ascontiguousarray(C), np.ascontiguousarray(S)


def _masks():
    p = np.arange(128)[:, None]
    f = np.arange(512)[None, :]
    mk = np.stack([(p <= f - 128 * j) for j in range(4)])
    return mk.astype(ml_dtypes.bfloat16)


def _swapmat(np16=np.float16):
    # out[m] = in[(m+64) % 128] under matmul(out, lhsT=sw, rhs=in)
    sw = np.zeros((128, 128), np16)
    m = np.arange(128)
    sw[(m + 64) % 128, m] = 1
    return sw


_NC_CACHE = {}


def _get_nc():
    key = (L, D, NH)
    if key not in _NC_CACHE:
        _NC_CACHE[key] = build_nc(L, D, NH, F16)
    return _NC_CACHE[key]


def make_in_maps(x, Wq, Wk, Wv, Wout, attn_scale, np16=np.float16):
    """Shard + lay out inputs for the 8 cores. Pure marshalling (no math)."""
    x = np.asarray(x, np.float32)
    Wq = np.asarray(Wq, np.float32)
    Wk = np.asarray(Wk, np.float32)
    Wv = np.asarray(Wv, np.float32)
    Wout = np.asarray(Wout, np.float32)
    scale = float(np.asarray(attn_scale))

    perm = np.concatenate([np.arange(0, Dh, 2), np.arange(1, Dh, 2)])
    Wq_p = Wq[:, :, perm]
    Wk_p = Wk[:, :, perm]

    C, S = _rope_tables(L, np16)
    mk = _masks()
    sc = np.full((128, 1), scale, np.float32)

    xT = [np.ascontiguousarray(x[b].T).astype(np16) for b in range(B)]
    in_maps = []
    for core in range(N_CORES):
        b, hg = divmod(core, N_CORES // B)
        hs = slice(NH * hg, NH * hg + NH)
        in_maps.append({
            "xT": xT[b],
            "wq": np.ascontiguousarray(
                Wq_p[:, hs].reshape(D, NH * 128)).astype(np16),
            "wk": np.ascontiguousarray(
                Wk_p[:, hs].reshape(D, NH * 128)).astype(np16),
            "wv": np.ascontiguousarray(
                Wv[:, hs].reshape(D, NH * 128)).astype(np16),
            "wo": np.ascontiguousarray(
                Wout[512 * hg:512 * hg + 512]).astype(np16),
            "cs": C, "sn": S, "mk": mk, "sc": sc, "sw": _swapmat(np16),
        })
    return in_maps


def combine_results(results):
    """Sum the 4 partial [L, D] outputs per batch."""
    y = np.zeros((B, L, D), np.float32)
    for core, r in enumerate(results):
        b = core // (N_CORES // B)
        y[b] += np.asarray(r["y"], np.float32)
    return y


def kernel(x, Wq, Wk, Wv, Wout, attn_scale):
    from concourse.bass_utils import run_bass_kernel_spmd
    nc = _get_nc()
    in_maps = make_in_maps(x, Wq, Wk, Wv, Wout, attn_scale)
    res = run_bass_kernel_spmd(nc, in_maps, core_ids=list(range(N_CORES)))
    return combine_results(res.results)



# revision 2
# speedup vs baseline: 65.1757x; 65.1757x over previous
"""Trainium2 Bass kernel for nn_CausalAttn_24618752541290.

Causal attention: B=2, L=2048, D=2048, H=16 heads, Dh=128, with RoPE
(theta=5e5, interleaved pairs), QK L2-normalization, causal softmax with a
runtime scale, and an output projection.

Sharding (8 NeuronCores): data-parallel over batch (2) x tensor-parallel over
head groups (4 heads/core).  Core i handles batch i//4 and heads
[4*(i%4), 4*(i%4)+4).  Each core computes a partial [L, D] output
(row-parallel Wout over its head slice); the host sums the 4 partials per
batch.

All matmuls are f16 (fp8 DoubleRow was prototyped and rejected: this
attention is sharp, S_eff ~ 2, so q/k quantization noise lands directly on
the output — emulated rel err 6e-2 vs the 2e-2 budget).  Relative to the
first working version, the device program is restructured for PE occupancy
and lower per-instruction overhead:
  - rope's half-swap runs as two SBUF->SBUF DMAs instead of a PE
    permutation matmul (frees PE cycles and a PSUM pool).
  - the 1/||q|| partition broadcast is a K=1 ones-matmul into PSUM instead
    of an SBUF->DRAM->SBUF round trip (the denominator broadcast keeps the
    DRAM route: its DVE consumer already reads avP from PSUM and DVE can
    read only one PSUM operand).
  - q/k norm matmuls are emitted at chunk end, after the v-projection
    matmuls, so their scalar/DVE input chains are ready when the in-order
    PE reaches them.
  - attention emission is software-pipelined one block ahead (S(bk+1)
    before exp/dn/av(bk)) so PE never head-of-line blocks on the exp.
  - diagonal attention blocks only compute the causally-live column tail
    (saves ~15us of PE and shrinks the mask multiply to a 128-wide
    triangle).
  - x loads are one DMA per 4 contraction tiles instead of 16 per chunk;
    y stores one DMA per row block instead of 4.
"""

import os
import sys

for p in ("/opt/trn_rl_repo",):
    if p not in sys.path:
        sys.path.insert(0, p)

import numpy as np
import ml_dtypes

import concourse.bass as bass
import concourse.mybir as mybir
from concourse.tile import TileContext
from contextlib import ExitStack

B, L, D, H = 2, 2048, 2048, 16
Dh = 128
NH = 4              # heads per core
N_CORES = 8
EPS = 1e-6
THETA = 500000.0

F32 = mybir.dt.float32
F32R = mybir.dt.float32r
F16 = mybir.dt.float16
BF16 = mybir.dt.bfloat16

# ---------------------------------------------------------------------------
# workarounds for this container's walrus build (single-sem-wait limit);
# identical to v1.
# ---------------------------------------------------------------------------

def _split_drain_and_barrier(self, tick_clock, wait_clock):
    from concourse.vector_clock import ScopedClock
    import bass_rust

    drain_inst = self.nc.sync.drain()
    wait_clock.add_sem_waits(
        drain_inst.ins, ScopedClock({None: tick_clock.global_clock}))
    si = drain_inst.ins.sync_info
    if si is not None and si.on_wait is not None and len(si.on_wait) > 1:
        waits = list(si.on_wait)
        si.on_wait = waits[:1]
        for w in waits[1:]:
            extra = self.nc.sync.drain()
            esi = extra.ins.sync_info
            if esi is None:
                extra.ins.sync_info = bass_rust.SyncInfo(
                    on_wait=[w], on_update=[])
            else:
                esi.on_wait = [w]

    self.nc.all_engine_barrier()
    assert self.sems is not None
    popped = self.nc._tile_sem_poison_stack.pop()
    assert popped is self._sem_poison
    self.nc.clear_and_free_semaphores(list(self.sems.allocated().values()))
    self.nc.all_engine_barrier()


TileContext._drain_and_barrier = _split_drain_and_barrier


def _split_waits_json(mod: dict, max_waits: int = 1) -> dict:
    for fn in mod.get("functions", []):
        for bb in fn.get("blocks", []):
            out = []
            for inst in bb.get("instructions", []):
                si = inst.get("sync_info")
                waits = (si or {}).get("on_wait") or []
                if len(waits) > max_waits:
                    extra, keep = waits[:-max_waits], waits[-max_waits:]
                    for k_, w in enumerate(extra):
                        out.append({
                            "debug": inst.get("debug", 0),
                            "engine": inst["engine"],
                            "ins": [],
                            "name": f"{inst['name']}.wsplit{k_}",
                            "opcode": "EventSemaphore",
                            "outs": [],
                            "sync_info": {"on_update": [], "on_wait": [w]},
                        })
                    si["on_wait"] = keep
                out.append(inst)
            bb["instructions"] = out
    return mod


_orig_to_json_bytes = bass.Bass.to_json_bytes


def _to_json_bytes_split(self):
    import orjson
    mod = orjson.loads(_orig_to_json_bytes(self))
    _split_waits_json(mod)
    return orjson.dumps(mod)


bass.Bass.to_json_bytes = _to_json_bytes_split


# ---------------------------------------------------------------------------
# device program
# ---------------------------------------------------------------------------

def build_nc(l=L, d=D, nh=NH, reps=1):
    KT = d // 128          # contraction tiles over D
    CH = l // 512          # L chunks of 512
    NB = l // 128          # L blocks of 128
    nc = bass.Bass()

    xT_d = nc.dram_tensor("xT", [d, l], F16, kind="ExternalInput")
    wq_d = nc.dram_tensor("wq", [d, nh * 128], F16, kind="ExternalInput")
    wk_d = nc.dram_tensor("wk", [d, nh * 128], F16, kind="ExternalInput")
    wv_d = nc.dram_tensor("wv", [d, nh * 128], F16, kind="ExternalInput")
    wo_d = nc.dram_tensor("wo", [nh * 128, d], F16, kind="ExternalInput")
    cs_d = nc.dram_tensor("cs", [128, l], F16, kind="ExternalInput")
    sn_d = nc.dram_tensor("sn", [128, l], F16, kind="ExternalInput")
    mk_d = nc.dram_tensor("mk", [4, 128, 512], BF16, kind="ExternalInput")
    sc_d = nc.dram_tensor("sc", [128, 1], F32, kind="ExternalInput")
    y_d = nc.dram_tensor("y", [l, d], F16, kind="ExternalOutput")

    xT_r = xT_d.rearrange("(kt p) l -> p kt l", p=128)
    wq_r = wq_d.rearrange("(kt p) m -> p kt m", p=128)
    wk_r = wk_d.rearrange("(kt p) m -> p kt m", p=128)
    wv_r = wv_d.rearrange("(kt p) m -> p kt m", p=128)
    wo_r = wo_d.rearrange("(h p) d -> p h d", p=128)
    mk_r = mk_d.rearrange("j p f -> p j f")
    y_r = y_d.rearrange("(m p) d -> p m d", p=128)

    Exp = mybir.ActivationFunctionType.Exp
    Sqrt = mybir.ActivationFunctionType.Sqrt

    with TileContext(nc) as tc, ExitStack() as top:
        persist = top.enter_context(tc.tile_pool(name="persist", bufs=1))
        scratch = top.enter_context(tc.tile_pool(name="scratch", bufs=3))
        tiny = top.enter_context(tc.tile_pool(name="tiny", bufs=3))
        drp = top.enter_context(tc.tile_pool(name="drp", bufs=3, space="DRAM"))

        def bcast_ap(ap, n=128):
            return bass.AP(ap.tensor, ap.offset, [[0, n]] + list(ap.ap[1:]))

        # persistent SBUF
        qT_sb = persist.tile([128, nh, l], F16)
        kT_sb = persist.tile([128, nh, l], F16)
        outT_sb = persist.tile([128, nh, l], F16)
        v_sb = persist.tile([128, NB, nh * 128], BF16)
        crk_sb = persist.tile([128, nh, NB], F32)
        rk_sb = persist.tile([128, nh, NB], F32)
        mk_sb = persist.tile([128, 4, 512], BF16)
        sc_sb = persist.tile([128, 1], F32)
        ones16 = persist.tile([128, 1], F16)
        onesbf = persist.tile([128, 1], BF16)
        onesrow = persist.tile([1, 128], F16)    # K=1 bcast lhsT (q-norm)

        nc.sync.dma_start(mk_sb[:], mk_r[:])
        nc.sync.dma_start(sc_sb[:], sc_d[:])
        nc.vector.memset(ones16[:], 1.0)
        nc.vector.memset(onesbf[:], 1.0)
        nc.vector.memset(onesrow[:], 1.0)

        import contextlib
        rep_cm = tc.For_i(0, reps, 1) if reps > 1 else contextlib.nullcontext()
        top.enter_context(rep_cm)

        # ---------------- q/k/v projections + rope + norms ----------------
        with ExitStack() as qk_phase:
            pool_w = qk_phase.enter_context(tc.tile_pool(name="pool_w", bufs=1))
            pool_cs = qk_phase.enter_context(tc.tile_pool(name="pool_cs", bufs=1))
            xcpool = qk_phase.enter_context(tc.tile_pool(name="xcpool", bufs=2))
            psq = qk_phase.enter_context(
                tc.tile_pool(name="psq", bufs=3, space="PSUM"))
            psb = qk_phase.enter_context(
                tc.tile_pool(name="psb", bufs=1, space="PSUM"))
            psk = qk_phase.enter_context(
                tc.tile_pool(name="psk", bufs=2, space="PSUM"))
            wq_sb = pool_w.tile([128, KT, nh * 128], F16)
            wk_sb = pool_w.tile([128, KT, nh * 128], F16)
            wv_sb = pool_w.tile([128, KT, nh * 128], F16)
            cs_sb = pool_cs.tile([128, l], F16)
            sn_sb = pool_cs.tile([128, l], F16)
            for kt4 in range(0, KT, 4):
                nc.sync.dma_start(wq_sb[:, kt4:kt4 + 4, :],
                                  wq_r[:, kt4:kt4 + 4, :])
            for kt4 in range(0, KT, 4):
                nc.sync.dma_start(wk_sb[:, kt4:kt4 + 4, :],
                                  wk_r[:, kt4:kt4 + 4, :])
            for kt4 in range(0, KT, 4):
                nc.sync.dma_start(wv_sb[:, kt4:kt4 + 4, :],
                                  wv_r[:, kt4:kt4 + 4, :])
            nc.sync.dma_start(cs_sb[:], cs_d[:])
            nc.sync.dma_start(sn_sb[:], sn_d[:])

            for c in range(CH):
                cs_ = slice(c * 512, (c + 1) * 512)
                xc = xcpool.tile([128, KT, 512], F16, tag="xc")
                for kt4 in range(0, KT, 4):
                    nc.sync.dma_start(xc[:, kt4:kt4 + 4, :],
                                      xT_r[:, kt4:kt4 + 4, cs_])

                # stage 1: q/k projections + rope chains (no norm matmuls yet)
                deferred = []
                for h in range(nh):
                    for is_k in (False, True):
                        w_sb = wk_sb if is_k else wq_sb
                        pP = psq.tile([128, 512], F32, tag="pP", name="pP")
                        for kt in range(KT):
                            nc.tensor.matmul(
                                pP[:],
                                lhsT=w_sb[:, kt, h * 128:(h + 1) * 128],
                                rhs=xc[:, kt, :],
                                start=(kt == 0), stop=(kt == KT - 1))
                        qf = scratch.tile([128, 512], F16, tag="qf")
                        nc.scalar.copy(qf[:], pP[:])
                        # swap the two Dh halves via SBUF->SBUF DMA
                        qsw = scratch.tile([128, 512], F16, tag="qsw")
                        nc.sync.dma_start(qsw[0:64, :], qf[64:128, :])
                        nc.sync.dma_start(qsw[64:128, :], qf[0:64, :])
                        qsq = scratch.tile([128, 512], F16, tag="qsq",
                                           bufs=5)
                        nc.vector.tensor_mul(out=qsq[:], in0=qf[:], in1=qf[:])
                        qc = scratch.tile([128, 512], F16, tag="qc")
                        nc.vector.tensor_mul(out=qc[:], in0=qf[:],
                                             in1=cs_sb[:, cs_])
                        qs = scratch.tile([128, 512], F16, tag="qs")
                        nc.vector.tensor_mul(out=qs[:], in0=qsw[:],
                                             in1=sn_sb[:, cs_])
                        if is_k:
                            # roped, unnormalized k straight into kT
                            nc.vector.tensor_add(
                                out=kT_sb[:, h, cs_], in0=qc[:], in1=qs[:])
                        else:
                            qr = scratch.tile([128, 512], F16, tag="qr",
                                              bufs=5)
                            nc.vector.tensor_add(out=qr[:], in0=qc[:],
                                                 in1=qs[:])
                        deferred.append((h, is_k, qsq, None if is_k else qr))

                # stage 2: v projections (covers the rope chains above)
                for j in range(4):
                    blk = 4 * c + j
                    vP = psq.tile([128, nh * 128], F32, tag="pP", name="vP")
                    for kt in range(KT):
                        nc.tensor.matmul(
                            vP[:],
                            lhsT=xc[:, kt, j * 128:(j + 1) * 128],
                            rhs=wv_sb[:, kt, :],
                            start=(kt == 0), stop=(kt == KT - 1))
                    nc.scalar.copy(v_sb[:, blk, :], vP[:])

                # stage 3: norm matmuls + q writes (inputs long since ready)
                for h, is_k, qsq, qr in deferred:
                    if is_k:
                        nk = psk.tile([128, 4], F32, tag="nk", name="nk")
                        for j in range(4):
                            nc.tensor.matmul(
                                nk[:, j:j + 1],
                                lhsT=qsq[:, j * 128:(j + 1) * 128],
                                rhs=ones16[:],
                                start=True, stop=True)
                        nks = tiny.tile([128, 4], F32, tag="nks")
                        nc.scalar.sqrt(nks[:], nk[:])
                        nke = tiny.tile([128, 4], F32, tag="nks")
                        nc.vector.tensor_scalar_add(nke[:], nks[:], EPS)
                        nc.vector.reciprocal(
                            rk_sb[:, h, c * 4:(c + 1) * 4], nke[:])
                    else:
                        nq = psb.tile([1, 512], F32, tag="nq", name="nq")
                        nc.tensor.matmul(nq[:], lhsT=ones16[:], rhs=qsq[:],
                                         start=True, stop=True)
                        sq = tiny.tile([1, 512], F32, tag="sq")
                        nc.scalar.activation(sq[:], nq[:], Sqrt)
                        sqe = tiny.tile([1, 512], F32, tag="sq")
                        nc.vector.tensor_scalar_add(sqe[:], sq[:], EPS)
                        rq = tiny.tile([1, 512], F16, tag="rq")
                        with nc.allow_low_precision(
                                reason="1/||q|| ~0.1, f16 rel err 5e-4"):
                            nc.vector.reciprocal(rq[:], sqe[:])
                        rqb = psb.tile([128, 512], F32, tag="rqb", name="rqb")
                        nc.tensor.matmul(rqb[:], lhsT=onesrow[:],
                                         rhs=rq[:], start=True, stop=True)
                        nc.vector.tensor_mul(
                            out=qT_sb[:, h, cs_], in0=qr[:], in1=rqb[:])
            # crk = attn_scale * rk  (broadcast scale along free dim)
            nc.vector.tensor_tensor(
                out=crk_sb.rearrange("p h b -> p (h b)"),
                in0=rk_sb.rearrange("p h b -> p (h b)"),
                in1=sc_sb[:].to_broadcast((128, nh * NB)),
                op=mybir.AluOpType.mult)

        # ---------------- attention ----------------
        with ExitStack() as va_phase:
            pool_wo = va_phase.enter_context(tc.tile_pool(name="pool_wo", bufs=1))
            wo_sb = pool_wo.tile([128, nh, d], F16)
            nc.sync.dma_start(wo_sb[:], wo_r[:])
            with ExitStack() as at_phase:
                pss = at_phase.enter_context(
                    tc.tile_pool(name="pss", bufs=3, space="PSUM"))
                psav = at_phase.enter_context(
                    tc.tile_pool(name="psav", bufs=2, space="PSUM"))
                psdn = at_phase.enter_context(
                    tc.tile_pool(name="psdn", bufs=2, space="PSUM"))
                ppool = at_phase.enter_context(tc.tile_pool(name="ppool", bufs=6))

                for h in range(nh):
                    for c in range(CH):
                        cs_ = slice(c * 512, (c + 1) * 512)
                        nbk = 4 * c + 4
                        avP = psav.tile([128, 512], F32, tag="avP", name="avP")
                        dnT = psdn.tile([1, 512], F32, tag="dnP", name="dnT")
                        dnP = dnT[:]

                        def consume(bk, sP):
                            # diagonal blocks: columns below q=128*bk are
                            # causally dead — compute only the live tail.
                            j = max(bk - 4 * c, 0)
                            o, w = 128 * j, 512 - 128 * j
                            Pb = ppool.tile([128, 512], BF16, tag="Pb",
                                            name="Pb")
                            nc.scalar.activation(
                                Pb[:, :w], sP[:, :w], Exp,
                                scale=crk_sb[:, h, bk:bk + 1])
                            if bk >= 4 * c:
                                nc.vector.tensor_mul(
                                    out=Pb[:, 0:128], in0=Pb[:, 0:128],
                                    in1=mk_sb[:, 0, 0:128])
                            nc.tensor.matmul(dnT[:, o:512],
                                             lhsT=onesbf[:], rhs=Pb[:, :w],
                                             start=(bk == 0),
                                             stop=(bk == nbk - 1))
                            nc.tensor.matmul(
                                avP[:, o:512],
                                lhsT=v_sb[:, bk, h * 128:(h + 1) * 128],
                                rhs=Pb[:, :w],
                                start=(bk == 0), stop=(bk == nbk - 1))

                        prev = None
                        for bk in range(nbk):
                            j = max(bk - 4 * c, 0)
                            sP = pss.tile([128, 512], F32, tag="sP", name="sP")
                            nc.tensor.matmul(
                                sP[:, :512 - 128 * j],
                                lhsT=kT_sb[:, h, bk * 128:(bk + 1) * 128],
                                rhs=qT_sb[:, h, c * 512 + 128 * j:
                                          (c + 1) * 512],
                                start=True, stop=True)
                            if prev is not None:
                                consume(*prev)
                            prev = (bk, sP)
                        consume(*prev)

                        rdn = tiny.tile([1, 512], F32, tag="rdn")
                        nc.vector.reciprocal(rdn[:], dnP)
                        rdr = drp.tile([1, 512], F32, tag="rdr", name="rdr")
                        nc.sync.dma_start(rdr[:], rdn[:])
                        rdb = scratch.tile([128, 512], F32, tag="rdb")
                        nc.sync.dma_start(rdb[:], bcast_ap(rdr))
                        nc.vector.tensor_mul(
                            out=outT_sb[:, h, cs_], in0=avP[:], in1=rdb[:])

            # ---------------- output projection ----------------
            with ExitStack() as wo_phase:
                psy = wo_phase.enter_context(
                    tc.tile_pool(name="psy", bufs=1, space="PSUM"))
                ypool = wo_phase.enter_context(tc.tile_pool(name="ypool", bufs=2))
                ND = d // 512
                for m in range(NB):
                    yP = []
                    for n in range(ND):
                        yPn = psy.tile([128, 512], F32, tag=f"yP{n}",
                                       name="yPn")
                        yP.append(yPn)
                    for h in range(nh):
                        for n in range(ND):
                            nc.tensor.matmul(
                                yP[n][:],
                                lhsT=outT_sb[:, h, m * 128:(m + 1) * 128],
                                rhs=wo_sb[:, h, n * 512:(n + 1) * 512],
                                start=(h == 0), stop=(h == nh - 1))
                    ysb = ypool.tile([128, d], F16, tag="ysb")
                    for n in range(ND):
                        nc.vector.tensor_copy(
                            out=ysb[:, n * 512:(n + 1) * 512], in_=yP[n][:])
                    nc.sync.dma_start(y_r[:, m, :], ysb[:])

    return nc


# ---------------------------------------------------------------------------
# host-side input marshalling
# ---------------------------------------------------------------------------

def _rope_tables(l, np16):
    inv = 1.0 / (THETA ** (np.arange(0, Dh, 2, dtype=np.float64) / Dh))
    t = np.arange(l, dtype=np.float64)
    fr = np.outer(t, inv)                       # [l, 64]
    cos, sin = np.cos(fr).T, np.sin(fr).T       # [64, l]
    C = np.concatenate([cos, cos], axis=0).astype(np16)
    S = np.concatenate([-sin, sin], axis=0).astype(np16)
    return np.ascontiguousarray(C), np.ascontiguousarray(S)


def _masks():
    p = np.arange(128)[:, None]
    f = np.arange(512)[None, :]
    mk = np.stack([(p <= f - 128 * j) for j in range(4)])
    return mk.astype(ml_dtypes.bfloat16)


_NC_CACHE = {}


def _get_nc():
    key = (L, D, NH)
    if key not in _NC_CACHE:
        _NC_CACHE[key] = build_nc(L, D, NH)
    return _NC_CACHE[key]


def make_in_maps(x, Wq, Wk, Wv, Wout, attn_scale, np16=np.float16):
    """Shard + lay out inputs for the 8 cores. Pure marshalling (no math)."""
    x = np.asarray(x, np.float32)
    Wq = np.asarray(Wq, np.float32)
    Wk = np.asarray(Wk, np.float32)
    Wv = np.asarray(Wv, np.float32)
    Wout = np.asarray(Wout, np.float32)
    scale = float(np.asarray(attn_scale))

    perm = np.concatenate([np.arange(0, Dh, 2), np.arange(1, Dh, 2)])
    Wq_p = Wq[:, :, perm]
    Wk_p = Wk[:, :, perm]

    C, S = _rope_tables(L, np16)
    mk = _masks()
    sc = np.full((128, 1), scale, np.float32)

    xT = [np.ascontiguousarray(x[b].T).astype(np16) for b in range(B)]
    in_maps = []
    for core in range(N_CORES):
        b, hg = divmod(core, N_CORES // B)
        hs = slice(NH * hg, NH * hg + NH)
        in_maps.append({
            "xT": xT[b],
            "wq": np.ascontiguousarray(
                Wq_p[:, hs].reshape(D, NH * 128)).astype(np16),
            "wk": np.ascontiguousarray(
                Wk_p[:, hs].reshape(D, NH * 128)).astype(np16),
            "wv": np.ascontiguousarray(
                Wv[:, hs].reshape(D, NH * 128)).astype(np16),
            "wo": np.ascontiguousarray(
                Wout[512 * hg:512 * hg + 512]).astype(np16),
            "cs": C, "sn": S, "mk": mk, "sc": sc,
        })
    return in_maps


def combine_results(results):
    """Sum the 4 partial [L, D] outputs per batch."""
    y = np.zeros((B, L, D), np.float32)
    for core, r in enumerate(results):
        b = core // (N_CORES // B)
        y[b] += np.asarray(r["y"], np.float32)
    return y


def kernel(x, Wq, Wk, Wv, Wout, attn_scale):
    from concourse.bass_utils import run_bass_kernel_spmd
    nc = _get_nc()
    in_maps = make_in_maps(x, Wq, Wk, Wv, Wout, attn_scale)
    res = run_bass_kernel_spmd(nc, in_maps, core_ids=list(range(N_CORES)))
    return combine_results(res.results)
